# revision 35
# baseline (speedup 1.0000x reference)
"""Trainium2 Bass kernel for nn_Attention (sparse_attention, 8 NeuronCores).

Sharding: data-parallel over batch (4) x tensor-parallel over heads (2 groups
of 4 heads) = 8 cores. Each core computes attention for one batch and 4 heads
entirely in transposed (feature-major) layout, so no on-chip transposes are
needed. exp(attn_bias) is precomputed on the host in bf16, so on-chip softmax
is exp(S) * expB with no PSUM-blocking adds. Wo is row-sharded; each core
returns one bf16 partial per head-pair and the host reduces.

v2 notes: the whole weight/x pack is bf16 and DMA'd in just-in-time order
(kt-interleaved x/Wq/Wk first), bias streams per-(head, jt); PSUM
evacuations run on the otherwise-idle Pool engine (gpsimd) so ACT does only
the 32 exps; the softmax denominator broadcast uses gpsimd.partition_broadcast
instead of a PE ones-matmul; the epilogue runs in bf16 for 2x DVE throughput.
Attention is head-major within each pair so the first head's epilogue hides
under the second head's attention, and the final head's epilogue is split
into query-halves pipelined with the output projection.
"""

import os
import sys

for _p in ("/opt/trn_rl_repo", "/root/.axon_site/_ro/trn_rl_repo"):
    if os.path.isdir(_p) and _p not in sys.path:
        sys.path.append(_p)

import numpy as np

B, N, DIM, H, DH = 4, 1024, 512, 8, 64
SCALE = DH**-0.5
HL = 4  # heads per core
HDL = HL * DH  # 256 head-dims per core
NCORES = 8
NJT = N // 128  # 8 key-tiles
NKT = DIM // 128  # 4 contraction tiles

WPC = NKT * 1536 + NKT * 512 + 2  # 8194

_CACHE = {}


def _build(loop_iters=1):
    import concourse.tile as tile
    from concourse import bacc, mybir

    fp32 = mybir.dt.float32
    f32r = mybir.dt.float32r
    bf16 = mybir.dt.bfloat16

    Exp = mybir.ActivationFunctionType.Exp
    mult = mybir.AluOpType.mult

    nc = bacc.Bacc("TRN2", target_bir_lowering=False, debug=False, num_devices=NCORES)

    wpack = nc.dram_tensor("wpack", [128, WPC], bf16, kind="ExternalInput").ap()
    wo64 = nc.dram_tensor("wo64", [64, 4 * DIM], bf16, kind="ExternalInput").ap()
    expB = nc.dram_tensor("expB", [2, NJT, 128, 2 * N], bf16, kind="ExternalInput").ap()
    outT = nc.dram_tensor("outT", [2, 4, 128, N], bf16, kind="ExternalOutput").ap()

    from contextlib import ExitStack

    with tile.TileContext(nc) as tc, ExitStack() as stack:
        if loop_iters > 1:
            stack.enter_context(
                tc.For_i(0, loop_iters, 1, hint_engines=(mybir.EngineType.PE,))
            )
        with (
            tc.tile_pool(name="const", bufs=1) as cpool,
            tc.tile_pool(name="proj", bufs=1) as projpool,
            tc.tile_pool(name="bias", bufs=8) as biaspool,
            tc.tile_pool(name="etile", bufs=6) as epool,
            tc.tile_pool(name="work", bufs=2) as workpool,
            tc.tile_pool(name="psA", bufs=2, space="PSUM") as psA,
            tc.tile_pool(name="psB", bufs=2, space="PSUM") as psB,
        ):
            wp_sb = cpool.tile([128, WPC], bf16)
            state = {}

            # ---- weight-pack views ----
            def xT_kt(kt, lo, size):
                return wp_sb[:, kt * 1536 + lo : kt * 1536 + lo + size]

            def wqk(which, kt, mt):  # 0=q 1=k; [128, 128] slice for pair mt
                base = kt * 1536 + 1024 + which * 256 + mt * 128
                return wp_sb[:, base : base + 128]

            def wv_kt(kt):  # [128, 256]
                base = NKT * 1536 + kt * 512
                return wp_sb[:, base : base + 256]

            def wg_kt(kt, mt):  # [128, 128]
                base = NKT * 1536 + kt * 512 + 256 + mt * 128
                return wp_sb[:, base : base + 128]

            # Wo stored head-half-major at partition 0: [64, (2p+h)*512+d]
            wo_sb = cpool.tile([64, 4 * DIM], bf16, tag="wo64")
            bgf_sb = cpool.tile([128, 2], fp32, tag="bgf")

            # ---- DMA: x/Wq/Wk interleaved by kt so the PE starts early ----
            for kt in range(NKT):
                nc.sync.dma_start(
                    wp_sb[:, kt * 1536 : (kt + 1) * 1536],
                    wpack[:, kt * 1536 : (kt + 1) * 1536],
                )
            # Wv slices first (vproj starts early), then bias, Wg, Wo
            for kt in range(NKT):
                vg = NKT * 1536 + kt * 512
                nc.sync.dma_start(wp_sb[:, vg : vg + 256], wpack[:, vg : vg + 256])

            bias_tiles = {}

            def bias_dma(p, hh, jt):
                bt = biaspool.tile([128, N], bf16, tag="bias", name="bt")
                nc.sync.dma_start(bt[:], expB[p, jt, :, hh * N : (hh + 1) * N])
                bias_tiles[(p, hh, jt)] = bt

            bias_dma(0, 0, 0)
            bias_dma(0, 0, 1)
            for kt in range(NKT):
                vg = NKT * 1536 + kt * 512
                nc.sync.dma_start(
                    wp_sb[:, vg + 256 : vg + 512], wpack[:, vg + 256 : vg + 512]
                )
            bias_dma(0, 0, 2)
            nc.sync.dma_start(wo_sb[:], wo64)
            nc.sync.dma_start(wp_sb[:, 8192:WPC], wpack[:, 8192:WPC])  # bg
            bias_dma(0, 0, 3)
            ones_sb = cpool.tile([1, 64], bf16, tag="ones")

            # ---- projections ----
            qT_sb = [projpool.tile([128, N], bf16, tag=f"qT{m}", name=f"qT{m}") for m in range(2)]
            kT_sb = [projpool.tile([128, N], bf16, tag=f"kT{m}", name=f"kT{m}") for m in range(2)]
            gT_sb = [projpool.tile([128, N], bf16, tag=f"gT{m}", name=f"gT{m}") for m in range(2)]
            # partition-0 staged copy of each pair's second-head gates (the
            # BIR verifier requires TT operands to share a start partition;
            # only DMA can shift partitions)
            gTh1 = [projpool.tile([64, N], bf16, tag=f"gTh1_{m}", name=f"gTh1_{m}") for m in range(2)]

            def proj_mm(which, dst, mt, half, evac="gpsimd"):
                """Half of a q/k/g projection (2 of 4 kt tiles); the second
                half evacuates. Splitting keeps per-unit background PE work
                under ~900ns so the exp pipeline is never starved."""
                if half == 0:
                    state[("pps", which, mt)] = psA.tile(
                        [128, N], fp32, tag="big", name="ps"
                    )
                ps = state[("pps", which, mt)]
                for kt in (0, 1) if half == 0 else (2, 3):
                    lhsT = wqk(which, kt, mt) if which < 2 else wg_kt(kt, mt)
                    for ih in range(2):
                        nc.tensor.matmul(
                            ps[:, ih * 512 : ih * 512 + 512],
                            lhsT,
                            xT_kt(kt, ih * 512, 512),
                            start=(kt == 0),
                            stop=(kt == NKT - 1),
                        )
                if half == 0:
                    return
                if which == 2:
                    nc.vector.tensor_scalar_add(dst[mt][:], ps[:], bgf_sb[:, mt : mt + 1])
                elif evac == "scalar":
                    nc.scalar.copy(dst[mt][:], ps[:])
                else:
                    nc.vector.tensor_copy(dst[mt][:], ps[:])

            def proj(which, dst, mt, evac="gpsimd"):  # which: 0=q 1=k 2=g
                proj_mm(which, dst, mt, 0, evac)
                proj_mm(which, dst, mt, 1, evac)

            # v natural [token, d] + ones column per head (bf16)
            vhat_all = projpool.tile([128, NJT * HL * 65], bf16, tag="vhat")
            ones_view = vhat_all[:].rearrange(
                "p (j h c) -> p j h c", j=NJT, c=65
            )[:, :, :, 64:65]
            def vproj(jt):
                vv = vhat_all[:, jt * HL * 65 : (jt + 1) * HL * 65].rearrange(
                    "p (h c) -> p h c", h=HL
                )
                ps2 = psA.tile([128, HDL], fp32, tag="big", name="ps2")
                for kt in range(NKT):
                    nc.tensor.matmul(
                        ps2[:],
                        xT_kt(kt, jt * 128, 128),
                        wv_kt(kt),
                        start=(kt == 0),
                        stop=(kt == NKT - 1),
                    )
                nc.vector.tensor_copy(
                    vv[:, :, 0:64], ps2[:].rearrange("p (h c) -> p h c", h=HL)
                )

            # PE warm-up: dummy matmuls ramp the tensor engine to full
            # clock while the first weight DMAs are still in flight
            warm = cpool.tile([1, 512], bf16, tag="warm")
            nc.vector.memset(warm[:], 0.0)
            wps = psA.tile([1, 512], fp32, tag="big", name="warmps")
            for _ in range(10):
                nc.tensor.matmul(wps[:], warm[0:1, 0:1], warm[:], start=True, stop=True)

            proj(0, qT_sb, 0, evac="scalar")
            proj(1, kT_sb, 0, evac="vector")

            nc.vector.memset(ones_view, 1.0)
            nc.vector.memset(ones_sb[:], 1.0)
            nc.vector.tensor_copy(bgf_sb[:], wp_sb[:, 8192:8194])

            # ---- shared state across pairs ----
            U_sb = {}  # (p, hh) -> sbuf [65, N]
            ug_h = [
                [workpool.tile([64, N], bf16, tag=f"ug{p}_{h}", name=f"ug{p}_{h}", bufs=1) for h in range(2)]
                for p in range(2)
            ]
            osb = [workpool.tile([128, 4 * N], bf16, tag=f"osb{p}", name=f"osb{p}", bufs=1) for p in range(2)]

            def attn_pair(p, background, tail=None):
                """Head-major (hh, jt) loop for pair p. The AV matmul is
                emitted a few units behind its QK so the in-order PE never
                waits on the ACT-exp / DVE-mult chain. Background thunks fill
                the remaining PE slack; `tail(hh)` fires right after the last
                AV of head hh is emitted."""
                uv = [
                    psB.tile([65, N], fp32, tag="uv", name=f"uv{p}_{i}")
                    for i in range(2)
                ]
                state[("uv", p)] = uv
                pend = []  # delayed AV: (jt, hh, e_tile)

                def flush_av():
                    jt0, hh0, e0 = pend.pop(0)
                    h = 2 * p + hh0
                    base = jt0 * HL * 65 + h * 65
                    for ih in range(2):
                        nc.tensor.matmul(
                            uv[hh0][:, ih * 512 : ih * 512 + 512],
                            vhat_all[:, base : base + 65],
                            e0[:, ih * 512 : ih * 512 + 512],
                            start=(jt0 == 0),
                            stop=(jt0 == NJT - 1),
                        )
                    if jt0 == NJT - 1 and tail is not None:
                        tail(hh0)

                for u in range(2 * NJT):
                    hh, jt = divmod(u, NJT)
                    # prefetch bias 4 units ahead (rolls into next pair)
                    nu = u + 4
                    if nu < 2 * NJT:
                        nh, nj = divmod(nu, NJT)
                        if (p, nh, nj) not in bias_tiles:
                            bias_dma(p, nh, nj)
                    elif p == 0:
                        nh, nj = divmod(nu - 2 * NJT, NJT)
                        if (1, nh, nj) not in bias_tiles:
                            bias_dma(1, nh, nj)
                    bt = bias_tiles[(p, hh, jt)]

                    st = psA.tile([128, N], fp32, tag="big", name=f"st{u}")
                    lhsT = kT_sb[p][hh * 64 : hh * 64 + 64, jt * 128 : jt * 128 + 128]
                    for ih in range(2):
                        nc.tensor.matmul(
                            st[:, ih * 512 : ih * 512 + 512],
                            lhsT,
                            qT_sb[p][hh * 64 : hh * 64 + 64, ih * 512 : ih * 512 + 512],
                            start=True,
                            stop=True,
                        )
                    e1 = epool.tile([128, N], bf16, tag="e1", name="e1")
                    nc.scalar.activation(e1[:], st[:], Exp)
                    e = epool.tile([128, N], bf16, tag="e", name="e")
                    nc.vector.tensor_tensor(out=e[:], in0=e1[:], in1=bt[:], op=mult)
                    pend.append((jt, hh, e))
                    if len(pend) > 3:
                        flush_av()
                    for th in background[u] if u < len(background) else ():
                        th()
                while pend:
                    flush_av()
                for sched in background[2 * NJT :]:
                    for th in sched:
                        th()
                if p == 0:
                    # evacuate accumulators so the psB slots free for pair 1;
                    # the denominator row moves to partition 0 via DMA so the
                    # downstream reciprocal stays partition-aligned
                    for hh in range(2):
                        U = workpool.tile([65, N], bf16, tag="U", name=f"U{p}_{hh}", bufs=4)
                        nc.scalar.copy(U[:], uv[hh][:])
                        den = workpool.tile([1, N], bf16, tag="den", name=f"den{hh}", bufs=2)
                        nc.sync.dma_start(den[:], U[64:65, :])
                        U_sb[(p, hh)] = U
                        state[("den", p, hh)] = den

            def epi_rec(p, hh):
                """Reciprocal of the softmax denominator (DVE) + partition
                broadcast (Pool), full query width."""
                src = state[("den", p, hh)][0:1, :] if p == 0 else state[("uv", p)][hh][64:65, :]
                rec = workpool.tile([1, N], bf16, tag="rec", name="rec", bufs=4)
                with nc.allow_low_precision(reason="softmax denom, tol 2e-2"):
                    nc.vector.reciprocal(rec[:], src[:])
                bc = workpool.tile([64, N], bf16, tag="bc", name="bc", bufs=4)
                nc.gpsimd.partition_broadcast(bc[:], rec[0:1, :])
                state[("bc", p, hh)] = bc

            def epi_mul(p, hh):
                """Gate + denominator application (DVE), full query width.
                All operands start at partition 0."""
                src = U_sb[(p, hh)] if p == 0 else state[("uv", p)][hh]
                gt = gT_sb[p][0:64, :] if hh == 0 else gTh1[p][:]
                gs = workpool.tile([64, N], bf16, tag="gs", name="gs", bufs=4)
                nc.vector.tensor_tensor(
                    out=gs[:], in0=state[("bc", p, hh)][:], in1=gt, op=mult
                )
                nc.vector.tensor_tensor(
                    out=ug_h[p][hh][:], in0=src[0:64, :], in1=gs[:], op=mult
                )

            def outproj0_mt(mt):
                ps = psA.tile([128, N], fp32, tag="big", name="po")
                for ih in range(2):
                    sl = slice(ih * 512, ih * 512 + 512)
                    for h in range(2):
                        base = h * DIM + mt * 128
                        nc.tensor.matmul(
                            ps[:, sl],
                            wo_sb[:, base : base + 128],
                            ug_h[0][h][:, sl],
                            start=(h == 0),
                            stop=(h == 1),
                        )
                oslc = osb[0][:, mt * N : (mt + 1) * N]
                if mt % 2:
                    nc.vector.tensor_copy(oslc, ps[:])
                else:
                    nc.scalar.copy(oslc, ps[:])
                nc.sync.dma_start(outT[0, mt], oslc)

            # ---- pair 0: attention with v/q1/k1/g projections in the slack
            v = [lambda j=j: vproj(j) for j in range(NJT)]
            bg0 = [[v[j]] for j in range(NJT)] + [
                [lambda: proj_mm(0, qT_sb, 1, 0)],
                [lambda: proj_mm(0, qT_sb, 1, 1, evac="vector")],
                [lambda: proj_mm(1, kT_sb, 1, 0)],
                [lambda: proj_mm(1, kT_sb, 1, 1, evac="vector")],
                [lambda: proj_mm(2, gT_sb, 0, 0)],
                [lambda: (proj_mm(2, gT_sb, 0, 1),
                          nc.sync.dma_start(gTh1[0][:], gT_sb[0][64:128, :]))],
                [lambda: proj_mm(2, gT_sb, 1, 0)],
                [lambda: (proj_mm(2, gT_sb, 1, 1),
                          nc.sync.dma_start(gTh1[1][:], gT_sb[1][64:128, :]))],
            ]
            attn_pair(0, bg0)

            # ---- pair 1: pair-0 epilogue + output proj in the slack
            bg1 = [
                [lambda: epi_rec(0, 0)],
                [lambda: epi_mul(0, 0)],
                [lambda: epi_rec(0, 1)],
                [lambda: epi_mul(0, 1)],
                [],
                [lambda: outproj0_mt(0)],
                [],
                [lambda: outproj0_mt(1)],
                [],
                [lambda: outproj0_mt(2)],
                [],
                [lambda: outproj0_mt(3)],
                [lambda: epi_mul(1, 0)],
            ]

            def tail1(hh):
                if hh == 0:
                    epi_rec(1, 0)
                    return
                # Final head: phase-major epilogue split by query-half. The
                # denominator broadcast rides the (warm) PE; the ih0 gate/mult
                # chain runs on DVE while ih1's runs on Pool; the out
                # projection interleaves per query-half, and the four output
                # DMAs leave on four different queues to overlap issue.
                src_uv = state[("uv", 1)][1]
                recs = []
                for ih in range(2):
                    sl = slice(ih * 512, ih * 512 + 512)
                    rec = workpool.tile([1, 512], bf16, tag="rec", name="rec", bufs=4)
                    with nc.allow_low_precision(reason="softmax denom, tol 2e-2"):
                        nc.vector.reciprocal(rec[:], src_uv[64:65, sl])
                    recs.append(rec)
                bc = psB.tile([64, N], fp32, tag="uv", name="bctail")
                for ih in range(2):
                    nc.tensor.matmul(
                        bc[:, ih * 512 : ih * 512 + 512],
                        ones_sb[0:1, :],
                        recs[ih][0:1, :],
                        start=True,
                        stop=True,
                    )
                for ih in range(2):
                    sl = slice(ih * 512, ih * 512 + 512)
                    gs = workpool.tile([64, 512], bf16, tag="gs", name="gs", bufs=4)
                    nc.vector.tensor_tensor(
                        out=gs[:], in0=bc[:, sl], in1=gTh1[1][:, sl], op=mult
                    )
                    nc.vector.tensor_tensor(
                        out=ug_h[1][1][:, sl], in0=src_uv[0:64, sl], in1=gs[:], op=mult
                    )
                # out proj at [128,512] half granularity: four half-tiles
                # in flight across BOTH psum pools so all matmuls fire as soon
                # as each ug half lands; evacs spread over ACT/DVE/Pool and
                # the output DMAs alternate two queues.
                dma_engs = [nc.scalar, nc.sync]
                ev_engs = {
                    (0): lambda o, s: nc.scalar.copy(o, s),
                    (1): lambda o, s: nc.vector.tensor_copy(o, s),
                    (2): lambda o, s: nc.scalar.copy(o, s),
                    (3): lambda o, s: nc.vector.tensor_copy(o, s),
                }

                def po_half(mt, ih):
                    sl = slice(ih * 512, ih * 512 + 512)
                    if mt < 2:
                        ps = psA.tile([128, 512], fp32, tag="big", name=f"po1_{mt}_{ih}")
                    else:
                        ps = psB.tile([128, 512], fp32, tag="uv", name=f"po1_{mt}_{ih}")
                    for h in range(2):
                        base = (2 + h) * DIM + mt * 128
                        nc.tensor.matmul(
                            ps[:],
                            wo_sb[:, base : base + 128],
                            ug_h[1][h][:, sl],
                            start=(h == 0),
                            stop=(h == 1),
                        )
                    oslc = osb[1][:, mt * N + ih * 512 : mt * N + ih * 512 + 512]
                    ev_engs[mt](oslc, ps[:])
                    dma_engs[mt % 2].dma_start(outT[1, mt, :, sl], oslc)

                for ih in range(2):
                    for mt in range(4):
                        po_half(mt, ih)

            attn_pair(1, bg1, tail=tail1)

    nc.compile()
    return nc


def _shard_inputs(x, attn_bias, Wq, Wkv, Wg, bg, Wo):
    """Build per-core input maps (host-side layout prep)."""
    import ml_dtypes

    bf = ml_dtypes.bfloat16

    def kblocks(w):  # [512, F] -> [NKT, 128, F]
        return w.reshape(NKT, 128, w.shape[1])

    in_maps = []
    for d in range(NCORES):
        b, g = d // 2, d % 2
        cs = slice(g * HDL, (g + 1) * HDL)
        xTh = np.ascontiguousarray(x[b].T)  # [512, 1024]

        # expB [2, NJT, 128, 2N]: (pair, jt, key-row, hh*N + query)
        ab = attn_bias[b, g * HL : (g + 1) * HL]  # [4, i, j]
        abT = ab.transpose(0, 2, 1)  # [4, j, i]
        eb = np.exp(abT).astype(bf).reshape(2, 2, NJT, 128, N)  # [p, hh, jt, jrow, i]
        expB = np.ascontiguousarray(eb.transpose(0, 2, 3, 1, 4)).reshape(
            2, NJT, 128, 2 * N
        )

        xk = kblocks(xTh)  # [4, 128, 1024]
        wqk_ = kblocks(np.ascontiguousarray(Wq[:, cs]) * SCALE)  # [4,128,256]
        wkk = kblocks(np.ascontiguousarray(Wkv[:, g * HDL : (g + 1) * HDL]))
        wvk = kblocks(
            np.ascontiguousarray(Wkv[:, H * DH + g * HDL : H * DH + (g + 1) * HDL])
        )
        wgk = kblocks(np.ascontiguousarray(Wg[:, cs]))

        wpack = np.empty((128, WPC), np.float32)
        for kt in range(NKT):
            base = kt * 1536
            wpack[:, base : base + 1024] = xk[kt]
            wpack[:, base + 1024 : base + 1280] = wqk_[kt]
            wpack[:, base + 1280 : base + 1536] = wkk[kt]
            vg = NKT * 1536 + kt * 512
            wpack[:, vg : vg + 256] = wvk[kt]
            wpack[:, vg + 256 : vg + 512] = wgk[kt]
        wpack[:, 8192:8194] = bg[cs].reshape(2, 128).T
        # Wo head-half-major at partition 0: [64, (2p+h)*512 + d]
        wo64 = np.ascontiguousarray(
            Wo[cs, :].reshape(4, 64, DIM).transpose(1, 0, 2).reshape(64, 4 * DIM)
        )
        in_maps.append({"wpack": wpack.astype(bf), "expB": expB, "wo64": wo64.astype(bf)})
    return in_maps


def _unshard(results, bo):
    out = np.empty((B, N, DIM), dtype=np.float32)
    for b in range(B):
        acc = results[2 * b]["outT"].astype(np.float32).sum(axis=0) + results[
            2 * b + 1
        ]["outT"].astype(np.float32).sum(axis=0)
        out[b] = acc.reshape(DIM, N).T + bo[None, :]
    return out


def kernel(x, mask, attn_bias, Wq, Wkv, Wg, bg, Wo, bo):
    """Full inputs in, full output out. mask is all-ones by construction."""
    from concourse.bass_utils import run_bass_kernel_spmd

    x = np.asarray(x, dtype=np.float32)
    attn_bias = np.asarray(attn_bias, dtype=np.float32)
    Wq = np.asarray(Wq, dtype=np.float32)
    Wkv = np.asarray(Wkv, dtype=np.float32)
    Wg = np.asarray(Wg, dtype=np.float32)
    bg = np.asarray(bg, dtype=np.float32)
    Wo = np.asarray(Wo, dtype=np.float32)
    bo = np.asarray(bo, dtype=np.float32)

    if "nc" not in _CACHE:
        _CACHE["nc"] = _build()
    in_maps = _shard_inputs(x, attn_bias, Wq, Wkv, Wg, bg, Wo)
    res = run_bass_kernel_spmd(_CACHE["nc"], in_maps, core_ids=list(range(NCORES)))
    return _unshard(res.results, bo)
